# revision 1
# baseline (speedup 1.0000x reference)
"""Trainium2 Bass kernel for nn_Attention_65747359367242.

Per-batch tanh-attention with head-mean:
  Q = x@Wq+bq, K = cond@Wk+bk, V = cond@Wv+bv   (4 heads of 32 dims)
  S_h = Q_h K_h^T / sqrt(128)
  A   = mean_h tanh(mask + S_h)
  out = A @ V

Sharding: pure data-parallel, batch b -> core b (B=8, 8 cores). No collectives.

Device strategy per core (transposed orientation: scores S^T[m, n]):
  - host feeds x^T, cond^T, mask^T (bf16) + prescaled weights
  - Q^T/K^T/V computed on device via small matmuls (biases added as rank-1
    K=1 matmuls accumulating into the same PSUM)
  - main loop over (n-chunk 512, m-tile 128):
      * mask^T tile is injected into 4 PSUM banks (one per head) with
        identity-copy matmuls at the 4 diagonal 32x32 tile positions
        (start=True on the first -> clears bank, sets has_written)
      * 16 score matmuls (4 heads x 4 m-subtiles, K=32) packed at the 16
        32x32 tile positions accumulate S_h^T on top -> PSUM = mask + S_h
      * ScalarE tanh PSUM -> SBUF bf16 per head
      * VectorE sums the 4 heads (scale 1/4 folded into Wv/bv)
      * one matmul per m-tile accumulates out^T[d, n-chunk] += V'^T A^T
  - out^T streamed to DRAM; host transposes back.
"""

import math
import sys

import numpy as np

sys.path.insert(0, "/opt/trn_rl_repo")

import os
KSTAGE = int(os.environ.get("KSTAGE", "0"))  # 0 = full kernel
KSKIP = set(os.environ.get("KSKIP", "").split(","))
KREP = int(os.environ.get("KREP", "1"))  # on-device repeats of main loop

B, N, D = 8, 2048, 128
H, DH = 4, 32
NCH = 512            # n-chunk (free dim of score tiles / psum bank)
N_NCH = N // NCH     # 4
N_MT = N // 128      # 16 m-tiles

_NC_CACHE = {}


def _build_nc():
    from concourse import bass, tile
    from concourse.tile import add_dep_helper

    mybir = sys.modules["concourse.mybir"]
    f32 = mybir.dt.float32
    bf16 = mybir.dt.bfloat16
    TANH = mybir.ActivationFunctionType.Tanh

    nc = bass.Bass()

    xT = nc.declare_dram_parameter("xT", [D, N], bf16, isOutput=False)
    condT = nc.declare_dram_parameter("condT", [D, N], bf16, isOutput=False)
    maskT = nc.declare_dram_parameter("maskT", [N, N], bf16, isOutput=False)
    Wq = nc.declare_dram_parameter("Wq", [D, D], bf16, isOutput=False)
    Wk = nc.declare_dram_parameter("Wk", [D, D], bf16, isOutput=False)
    Wv4 = nc.declare_dram_parameter("Wv4", [D, D], bf16, isOutput=False)
    bq = nc.declare_dram_parameter("bq", [D, D], bf16, isOutput=False)
    bk = nc.declare_dram_parameter("bk", [D, D], bf16, isOutput=False)
    bv4 = nc.declare_dram_parameter("bv4", [D, D], bf16, isOutput=False)
    onesm = nc.declare_dram_parameter("onesm", [D, NCH], bf16, isOutput=False)
    eyed = nc.declare_dram_parameter("eyed", [D, 32], bf16, isOutput=False)
    eyef = nc.declare_dram_parameter("eyef", [D, D], bf16, isOutput=False)
    outT = [nc.declare_dram_parameter(f"outT{i}", [D, NCH], f32,
                                      isOutput=True) for i in range(N_NCH)]

    with tile.TileContext(nc) as tc:
        with (
            tc.tile_pool(name="const", bufs=1) as cpool,
            tc.tile_pool(name="proj", bufs=1) as projpool,
            tc.tile_pool(name="mask", bufs=64) as mpool,
            tc.tile_pool(name="th", bufs=8) as thpool,
            tc.tile_pool(name="at", bufs=3) as atpool,
            tc.tile_pool(name="osb", bufs=4 * KREP) as opool,
            tc.tile_pool(name="ps", bufs=3, space="PSUM") as pspool,
            tc.tile_pool(name="av", bufs=1, space="PSUM") as avpool,
            tc.tile_pool(name="gsb", bufs=70) as gsbpool,
        ):
            # ---- load constants / inputs ----
            wq_sb = cpool.tile([D, D], bf16, tag="wq")
            wk_sb = cpool.tile([D, D], bf16, tag="wk")
            wv_sb = cpool.tile([D, D], bf16, tag="wv")
            bq_sb = cpool.tile([D, D], bf16, tag="bq")
            bk_sb = cpool.tile([D, D], bf16, tag="bk")
            bv_sb = cpool.tile([D, D], bf16, tag="bv")
            ones_sb = cpool.tile([D, NCH], bf16, tag="ones")
            eyed_sb = cpool.tile([D, 32], bf16, tag="eyed")
            eyef_sb = cpool.tile([D, D], bf16, tag="eyef")
            xT_sb = cpool.tile([D, N], bf16, tag="xT")
            condT_sb = cpool.tile([D, N], bf16, tag="condT")

            # ldweights gates absorb DMA waits on the PE side (the Matmult
            # HW struct fits only one sync wait). They must be FULL-HEIGHT
            # [128, 1] loads: partial-height standalone ldweights before
            # tile_position matmuls hard-fault the PE
            # (NRT_EXEC_UNIT_UNRECOVERABLE).
            for sb_t, dr_t in [(wq_sb, Wq), (wk_sb, Wk), (wv_sb, Wv4),
                               (eyed_sb, eyed), (eyef_sb, eyef),
                               (xT_sb, xT), (condT_sb, condT)]:
                nc.sync.dma_start(out=sb_t[:], in_=dr_t[:])
                nc.tensor.ldweights(sb_t[:, 0:1])
            for sb_t, dr_t in [(bq_sb, bq), (bk_sb, bk), (bv_sb, bv4),
                               (ones_sb, onesm)]:
                nc.sync.dma_start(out=sb_t[:], in_=dr_t[:])
                nc.tensor.ldweights(sb_t[:, 0:1])

            # ---- projections ----
            # Q^T[d, n] = Wq'^T x^T + bq' x ones ; same for K^T. V[m, d] chunks.
            qT_sb = projpool.tile([D, N], bf16, tag="qT")
            kT_sb = projpool.tile([D, N], bf16, tag="kT")
            v_sb = projpool.tile([128, N], bf16, tag="v")  # chunk m at free 128m

            for c in range(N_NCH):
                sl = slice(c * NCH, (c + 1) * NCH)
                pq = pspool.tile([D, NCH], f32, tag="sc")
                nc.tensor.matmul(pq[:], wq_sb[:], xT_sb[:, sl],
                                 start=True, stop=False)
                nc.tensor.matmul(pq[:], bq_sb[:], ones_sb[:],
                                 start=False, stop=True)
                nc.vector.tensor_copy(qT_sb[:, sl], pq[:])

                pk = pspool.tile([D, NCH], f32, tag="sc")
                nc.tensor.matmul(pk[:], wk_sb[:], condT_sb[:, sl],
                                 start=True, stop=False)
                nc.tensor.matmul(pk[:], bk_sb[:], ones_sb[:],
                                 start=False, stop=True)
                nc.vector.tensor_copy(kT_sb[:, sl], pk[:])

            for t in range(N_MT):
                sl = slice(t * 128, (t + 1) * 128)
                pv = pspool.tile([128, D], f32, tag="sc")
                nc.tensor.matmul(pv[:], condT_sb[:, sl], wv_sb[:],
                                 start=True, stop=False)
                nc.tensor.matmul(pv[:], ones_sb[:, 0:128], bv_sb[:],
                                 start=False, stop=True)  # row0-padded rank-1
                nc.vector.tensor_copy(v_sb[:, sl], pv[:])

            # small ACT-written source tile for the gact2 gates
            actsrc = cpool.tile([1, 8], bf16, tag="actsrc")
            nc.scalar.copy(actsrc[0:1, 0:1], qT_sb[0:1, 0:1])

            # ---- main loop ----
            # The whole bf16 mask^T fits in SBUF (64 KiB/partition): issue
            # all 64 tile DMAs up-front into fresh slots. Fresh tiles carry
            # no WAR/WAW waits, which keeps every DMA within the sync-wait
    # slot budget, and gives maximal prefetch depth.
            n_nch_run = {1: 0, 2: 1, 3: 1}.get(KSTAGE, N_NCH)
            n_mt_run = {2: 1}.get(KSTAGE, N_MT)
            mk_tiles = {}
            tail_insts = []
            for ncg in range(N_NCH):
                for mt in range(N_MT):
                    mk = mpool.tile([128, NCH], bf16, tag="mk",
                                    name=f"mk_{ncg}_{mt}", bufs=64)
                    dmi = nc.sync.dma_start(
                        out=mk[:],
                        in_=maskT[mt * 128:(mt + 1) * 128,
                                  ncg * NCH:(ncg + 1) * NCH],
                    )
                    if ncg == N_NCH - 1 and mt >= N_MT - 8:
                        tail_insts.append(dmi)
                    mk_tiles[(ncg, mt)] = mk
            prev_tanh = []
            prev2_tanh = []
            prev_at = None
            prev_av_mm = None
            for rep in range(KREP):
              for ncg in range(n_nch_run):
                nsl = slice(ncg * NCH, (ncg + 1) * NCH)
                av = avpool.tile([D, NCH], f32, tag="av")
                for mt in range(n_mt_run):
                    mk = mk_tiles[(ncg, mt)]
                    # PE gates (tiny ldweights): g1 absorbs the mask DMA
                    # wait, g2 absorbs the ACT slot-release wait, so real
                    # matmuls carry at most one sync wait each.
                    nc.tensor.ldweights(mk[:, 0:1])
                    g2 = nc.tensor.ldweights(eyed_sb[:, 0:1])
                    if prev_tanh:
                        add_dep_helper(g2.ins, prev_tanh[0].ins,
                                       reason="sc release")
                    # paired-head PSUM tiles: heads (0,1) share tile A
                    # (2 banks), heads (2,3) share tile B -> one tanh per
                    # tile at FD=1024 instead of two at FD=512.
                    sc = [pspool.tile([128, 2 * NCH], f32, tag="sc",
                                      name=f"sc{p}") for p in range(2)]
                    # inject mask^T into each head's bank with a FULL-ARRAY
                    # identity matmul (tiled injects at other row groups
                    # racing the tiled score accumulation on the same PSUM
                    # addresses hard-fault the PE).
                    for h in range(H):
                        off = NCH * (h % 2)
                        mm = nc.tensor.matmul(
                            sc[h // 2][:, off:off + NCH], eyef_sb[:], mk[:],
                            start=True, stop=("scores" in KSKIP),
                            skip_group_check=True,
                        )
                        add_dep_helper(mm.ins, g2.ins, reason="gate order")
                    # packed per-head scores accumulate on top
                    for j in (range(0) if "scores" in KSKIP else range(4)):
                        qs = slice(32 * j, 32 * (j + 1))
                        moff = mt * 128 + 32 * j
                        for h in range(H):
                            hs = slice(32 * h, 32 * (h + 1))
                            off = NCH * (h % 2)
                            nc.tensor.matmul(
                                sc[h // 2][qs, off:off + NCH],
                                kT_sb[hs, moff:moff + 32],
                                qT_sb[hs, nsl],
                                start=False, stop=(j == 3),
                                tile_position=(32 * h, 32 * j),
                                skip_group_check=True,
                            )
                    # ACT gate: reads the previous group's A^T tile, which
                    # advances ACT's observed DVE clock past the th slot
                    # releases, keeping the tanhs within the sync-wait limit.
                    gates = []
                    if prev_at is not None:
                        gact = gsbpool.tile([1, 8], bf16, tag="gact",
                                            name=f"gact_{ncg}_{mt}", bufs=70)
                        ga = nc.scalar.copy(gact[0:1, 0:1], prev_at[0:1, 0:1])
                        gates.append(ga)
                    if prev2_tanh:
                        # absorb the th-slot WAW (tanh two groups back) at its
                        # max sem value so the real tanhs elide it
                        gact2 = gsbpool.tile([1, 8], bf16, tag="gact2",
                                             name=f"gact2_{ncg}_{mt}", bufs=70)
                        ga2 = nc.scalar.copy(gact2[0:1, 0:1], actsrc[0:1, 0:1])
                        for t in prev2_tanh:
                            add_dep_helper(ga2.ins, t.ins, reason="th waw")
                        gates.append(ga2)
                    # DVE gate: absorbs the at-slot-release (PE) wait so the
                    # head-sum adds carry at most one sync wait.
                    if prev_av_mm is not None:
                        gdve = gsbpool.tile([1, 8], bf16, tag="gdve",
                                            name=f"gdve_{ncg}_{mt}", bufs=70)
                        gd = nc.vector.tensor_copy(gdve[0:1, 0:1],
                                                   qT_sb[0:1, 0:1])
                        add_dep_helper(gd.ins, prev_av_mm.ins,
                                       reason="at release")
                    th = [thpool.tile([128, 2 * NCH], bf16, tag="th",
                                      name=f"th{p}") for p in range(2)]
                    prev2_tanh = prev_tanh
                    prev_tanh = []
                    for p in range(2):
                        act = nc.scalar.activation(th[p][:], sc[p][:], TANH)
                        prev_tanh.append(act)
                    # head sum
                    u = atpool.tile([128, NCH], bf16, tag="u")
                    w = atpool.tile([128, NCH], bf16, tag="w")
                    at = atpool.tile([128, NCH], bf16, tag="at")
                    nc.vector.tensor_add(u[:], th[0][:, 0:NCH],
                                         th[0][:, NCH:2 * NCH])
                    nc.vector.tensor_add(w[:], th[1][:, 0:NCH],
                                         th[1][:, NCH:2 * NCH])
                    nc.vector.tensor_add(at[:], u[:], w[:])
                    prev_at = at
                    # out^T[d, nch] += V'[mt]^T @ A^T[mt]
                    if "av" not in KSKIP:
                        prev_av_mm = nc.tensor.matmul(
                            av[:], v_sb[:, mt * 128:(mt + 1) * 128], at[:],
                            start=(mt == 0), stop=(mt == n_mt_run - 1),
                        )
                oT = opool.tile([D, NCH], f32, tag="oT", bufs=4 * KREP)
                if "av" in KSKIP:
                    cp = nc.vector.tensor_copy(oT[:], at[:])
                else:
                    cp = nc.vector.tensor_copy(oT[:], av[:])
                if rep == KREP - 1:
                    od = nc.gpsimd.dma_start(out=outT[ncg][:], in_=oT[:])
                    tail_insts.extend([cp, od])

            if KSTAGE == 1:
                for i in range(N_NCH):
                    oT = opool.tile([D, NCH], f32, tag="oT", bufs=4 * KREP)
                    cc = nc.vector.tensor_copy(
                        oT[:], qT_sb[:, i * NCH:(i + 1) * NCH])
                    od = nc.gpsimd.dma_start(out=outT[i][:], in_=oT[:])
                    tail_insts.extend([cc, od])
            tail_insts.extend(prev_tanh)
            if prev_av_mm is not None:
                tail_insts.append(prev_av_mm)
            for ti in tail_insts:
                nz = nc.sync.nop(nofuse=True, hint="predrain")
                add_dep_helper(nz.ins, ti.ins, reason="predrain absorb")

    return nc


def get_nc():
    if "nc" not in _NC_CACHE:
        _NC_CACHE["nc"] = _build_nc()
    return _NC_CACHE["nc"]


def _prep_in_maps(x, cond, attention_mask, Wq, bq, Wk, bk, Wv, bv):
    import ml_dtypes

    bf16 = ml_dtypes.bfloat16
    s = 1.0 / math.sqrt(128.0)

    Wq_s = (np.asarray(Wq, np.float32) * s).astype(bf16)
    Wk_b = np.asarray(Wk, np.float32).astype(bf16)
    Wv4 = (np.asarray(Wv, np.float32) * 0.25).astype(bf16)

    def _row0(vec):
        m = np.zeros((D, D), np.float32)
        m[0, :] = vec
        return m.astype(bf16)

    bq_s = _row0(np.asarray(bq, np.float32) * s)
    bk_b = _row0(np.asarray(bk, np.float32))
    bv4 = _row0(np.asarray(bv, np.float32) * 0.25)
    onesm = np.zeros((D, NCH), np.float32)
    onesm[0, :] = 1.0
    onesm = onesm.astype(bf16)
    eyed = np.tile(np.eye(32, dtype=np.float32), (4, 1)).astype(bf16)
    eyef = np.eye(D, dtype=np.float32).astype(bf16)

    x = np.asarray(x, np.float32)
    cond = np.asarray(cond, np.float32)
    attention_mask = np.asarray(attention_mask, np.float32)

    in_maps = []
    for i in range(B):
        in_maps.append({
            "xT": np.ascontiguousarray(x[i].T).astype(bf16),
            "condT": np.ascontiguousarray(cond[i].T).astype(bf16),
            "maskT": np.ascontiguousarray(attention_mask[i].T).astype(bf16),
            "Wq": Wq_s, "Wk": Wk_b, "Wv4": Wv4,
            "bq": bq_s, "bk": bk_b, "bv4": bv4,
            "onesm": onesm, "eyed": eyed, "eyef": eyef,
        })
    return in_maps


def run(x, cond, flags, attention_mask, Wq, bq, Wk, bk, Wv, bv,
        trace=False, tmpdir=None):
    """Returns (out [B,N,D] float32, exec_time_ns or None)."""
    from concourse.bass_utils import run_bass_kernel_spmd

    nc = get_nc()
    in_maps = _prep_in_maps(x, cond, attention_mask, Wq, bq, Wk, bk, Wv, bv)
    res = run_bass_kernel_spmd(
        nc, in_maps, core_ids=list(range(B)), trace=trace, tmpdir=tmpdir,
    )
    out = np.stack(
        [np.concatenate([np.asarray(r[f"outT{i}"], np.float32)
                         for i in range(N_NCH)], axis=1).T
         for r in res.results], axis=0
    )
    return out, res.exec_time_ns


def kernel(**inputs):
    out, _ = run(**inputs)
    return out



# revision 8
# speedup vs baseline: 1.5089x; 1.5089x over previous
"""Trainium2 Bass kernel for nn_Attention_65747359367242.

Per-batch tanh-attention with head-mean:
  Q = x@Wq+bq, K = cond@Wk+bk, V = cond@Wv+bv   (4 heads of 32 dims)
  S_h = Q_h K_h^T / sqrt(128)
  A   = mean_h tanh(mask + S_h)
  out = A @ V

Sharding: pure data-parallel, batch b -> core b (B=8, 8 cores). No collectives.

Device strategy per core (transposed orientation: scores S^T[m, n]):
  - host feeds x^T, cond^T, mask^T (bf16) + prescaled weights
  - Q^T/K^T/V computed on device via small matmuls (biases added as rank-1
    matmuls accumulating into the same PSUM)
  - main loop over (ncg: 4 n-chunks of 512) x (mt: 16 m-tiles of 128):
      * mask^T tile injected into 4 PSUM half-banks (one per head) via
        full-array identity matmuls (start=True clears, sets has_written)
      * 16 score matmuls (4 heads x 4 m-subtiles, K=32) packed at the 16
        32x32 tile positions accumulate S_h^T on top -> PSUM = mask + S_h
      * ScalarE tanh PSUM -> SBUF bf16, one per head-PAIR (FD=1024)
      * head-mean folded into AV by linearity: av[d, n] accumulates one
        matmul per head, moving operand = tanh slice (V' = Wv/4 prescaled)
  - out^T streamed to DRAM; host transposes back.

The ScalarE tanh stream (~128 us/core) is the theoretical bottleneck;
everything else (PE ~75 us, DVE ~25 us, DMA ~25 us) pipelines underneath.
"""

import math
import os
import sys

import numpy as np

sys.path.insert(0, "/opt/trn_rl_repo")

KREP = int(os.environ.get("KREP", "1"))  # on-device repeats of main loop

B, N, D = 8, 2048, 128
H, DH = 4, 32
NCH = 512            # n-chunk (free dim of score tiles / psum bank)
N_NCH = N // NCH     # 4
N_MT = N // 128      # 16 m-tiles

_NC_CACHE = {}


def _build_nc():
    from concourse import bass, tile
    from concourse.tile import add_dep_helper

    mybir = sys.modules["concourse.mybir"]
    f32 = mybir.dt.float32
    bf16 = mybir.dt.bfloat16
    TANH = mybir.ActivationFunctionType.Tanh

    nc = bass.Bass()

    xT = nc.declare_dram_parameter("xT", [D, N], bf16, isOutput=False)
    condT = nc.declare_dram_parameter("condT", [D, N], bf16, isOutput=False)
    maskT = nc.declare_dram_parameter("maskT", [N, N], bf16, isOutput=False)
    Wq = nc.declare_dram_parameter("Wq", [D, D], bf16, isOutput=False)
    Wk = nc.declare_dram_parameter("Wk", [D, D], bf16, isOutput=False)
    Wv4 = nc.declare_dram_parameter("Wv4", [D, D], bf16, isOutput=False)
    bq = nc.declare_dram_parameter("bq", [D, D], bf16, isOutput=False)
    bk = nc.declare_dram_parameter("bk", [D, D], bf16, isOutput=False)
    bv4 = nc.declare_dram_parameter("bv4", [D, D], bf16, isOutput=False)
    onesm = nc.declare_dram_parameter("onesm", [D, NCH], bf16, isOutput=False)
    eyef = nc.declare_dram_parameter("eyef", [D, D], bf16, isOutput=False)
    outT = [nc.declare_dram_parameter(f"outT{i}", [D, NCH], f32,
                                      isOutput=True) for i in range(N_NCH)]

    with tile.TileContext(nc) as tc:
        with (
            tc.tile_pool(name="const", bufs=1) as cpool,
            tc.tile_pool(name="proj", bufs=1) as projpool,
            tc.tile_pool(name="mask", bufs=16) as mpool,
            tc.tile_pool(name="th", bufs=6) as thpool,
            tc.tile_pool(name="osb", bufs=4 * KREP) as opool,
            tc.tile_pool(name="ps", bufs=3, space="PSUM") as pspool,
            tc.tile_pool(name="av", bufs=2, space="PSUM") as avpool,
            tc.tile_pool(name="gsb", bufs=66 * KREP) as gsbpool,
        ):
            # ---- load constants / inputs ----
            wq_sb = cpool.tile([D, D], bf16, tag="wq")
            wk_sb = cpool.tile([D, D], bf16, tag="wk")
            wv_sb = cpool.tile([D, D], bf16, tag="wv")
            bq_sb = cpool.tile([D, D], bf16, tag="bq")
            bk_sb = cpool.tile([D, D], bf16, tag="bk")
            bv_sb = cpool.tile([D, D], bf16, tag="bv")
            ones_sb = cpool.tile([D, NCH], bf16, tag="ones")
            eyef_sb = cpool.tile([D, D], bf16, tag="eyef")
            xT_sb = cpool.tile([D, N], bf16, tag="xT")
            condT_sb = cpool.tile([D, N], bf16, tag="condT")

            # ldweights gates absorb DMA waits on the PE side (the Matmult
            # HW struct fits only one sync wait). They must be FULL-HEIGHT
            # [128, 1] loads: partial-height standalone ldweights before
            # tile_position matmuls hard-fault the PE
            # (NRT_EXEC_UNIT_UNRECOVERABLE).
            for sb_t, dr_t in [(wq_sb, Wq), (wk_sb, Wk), (wv_sb, Wv4),
                               (eyef_sb, eyef), (xT_sb, xT),
                               (condT_sb, condT)]:
                nc.sync.dma_start(out=sb_t[:], in_=dr_t[:])
                nc.tensor.ldweights(sb_t[:, 0:1])
            for sb_t, dr_t in [(bq_sb, bq), (bk_sb, bk), (bv_sb, bv4),
                               (ones_sb, onesm)]:
                nc.sync.dma_start(out=sb_t[:], in_=dr_t[:])
                nc.tensor.ldweights(sb_t[:, 0:1])

            # ---- mask prefetch: 16 full-row DMAs [128, 2048] (4 KB/line) ----
            mk_tiles = []
            tail_insts = []
            for mt in range(N_MT):
                mk = mpool.tile([128, N], bf16, tag="mk",
                                name=f"mk_{mt}", bufs=16)
                dmi = nc.sync.dma_start(
                    out=mk[:], in_=maskT[mt * 128:(mt + 1) * 128, :])
                # gate: absorbs the DMA wait so inject matmuls carry only
                # their slot-release wait
                nc.tensor.ldweights(mk[:, 0:1])
                mk_tiles.append(mk)
                if mt >= N_MT - 8:
                    # predrain nops (below) absorb these lanes' final DMAHW
                    # ticks so the kernel-tail drain fits its wait slots
                    tail_insts.append(dmi)

            # ---- projections ----
            # Q^T[d, n] = Wq'^T x^T + bq' x ones ; same for K^T. V[m, d] chunks.
            qT_sb = projpool.tile([D, N], bf16, tag="qT")
            kT_sb = projpool.tile([D, N], bf16, tag="kT")
            v_sb = projpool.tile([128, N], bf16, tag="v")  # chunk m at free 128m

            for c in range(N_NCH):
                sl = slice(c * NCH, (c + 1) * NCH)
                pq = pspool.tile([D, NCH], f32, tag="sc")
                nc.tensor.matmul(pq[:], wq_sb[:], xT_sb[:, sl],
                                 start=True, stop=False)
                nc.tensor.matmul(pq[:], bq_sb[:], ones_sb[:],
                                 start=False, stop=True)
                nc.vector.tensor_copy(qT_sb[:, sl], pq[:])

                pk = pspool.tile([D, NCH], f32, tag="sc")
                nc.tensor.matmul(pk[:], wk_sb[:], condT_sb[:, sl],
                                 start=True, stop=False)
                nc.tensor.matmul(pk[:], bk_sb[:], ones_sb[:],
                                 start=False, stop=True)
                nc.vector.tensor_copy(kT_sb[:, sl], pk[:])

            for t in range(N_MT):
                sl = slice(t * 128, (t + 1) * 128)
                pv = pspool.tile([128, D], f32, tag="sc")
                nc.tensor.matmul(pv[:], condT_sb[:, sl], wv_sb[:],
                                 start=True, stop=False)
                nc.tensor.matmul(pv[:], ones_sb[:, 0:128], bv_sb[:],
                                 start=False, stop=True)  # row0-padded rank-1
                nc.vector.tensor_copy(v_sb[:, sl], pv[:])

            # small ACT-written source tile for the ACT gates below
            actsrc = cpool.tile([1, 8], bf16, tag="actsrc")
            nc.scalar.copy(actsrc[0:1, 0:1], qT_sb[0:1, 0:1])

            # ---- main loop ----
            prev_tanh = []
            prev2_tanh = []
            prev_av = []
            ot_copies = []
            for rep in range(KREP):
              for ncg in range(N_NCH):
                nsl = slice(ncg * NCH, (ncg + 1) * NCH)
                av = avpool.tile([D, NCH], f32, tag="av")
                if len(ot_copies) >= 2:
                    # PE gate: absorbs the av-slot release (DVE oT copy two
                    # ncg back) so the first AV matmul carries only its ACT
                    # wait. Full-height [128, 1] load (see note above).
                    gpe = nc.tensor.ldweights(v_sb[:, 0:1])
                    add_dep_helper(gpe.ins, ot_copies[-2].ins,
                                   reason="av slot release")
                for mt in range(N_MT):
                    mk = mk_tiles[mt]
                    # ACT gate: absorbs the th-slot WAW (tanh two groups
                    # back, same-engine completion wait) so the real tanhs
                    # carry only their PE wait. Walrus rejects >1 sync wait
                    # per Activation.
                    gates = []
                    if prev2_tanh:
                        gact = gsbpool.tile([1, 8], bf16, tag="gact",
                                            name=f"gact_{rep}_{ncg}_{mt}",
                                            bufs=66 * KREP)
                        ga = nc.scalar.copy(gact[0:1, 0:1], actsrc[0:1, 0:1])
                        for t_ in prev2_tanh:
                            add_dep_helper(ga.ins, t_.ins, reason="th waw")
                        gates.append(ga)
                    # paired-head PSUM tiles: heads (0,1) share tile A
                    # (2 banks), heads (2,3) share tile B -> one tanh per
                    # tile at FD=1024 instead of two at FD=512.
                    sc = [pspool.tile([128, 2 * NCH], f32, tag="sc",
                                      name=f"sc{p}") for p in range(2)]
                    # inject mask^T into each head's bank with a FULL-ARRAY
                    # identity matmul (tiled injects at other row groups
                    # racing the tiled score accumulation on the same PSUM
                    # addresses hard-fault the PE).
                    for h in range(H):
                        off = NCH * (h % 2)
                        nc.tensor.matmul(
                            sc[h // 2][:, off:off + NCH], eyef_sb[:],
                            mk[:, nsl],
                            start=True, stop=False,
                            skip_group_check=True,
                        )
                    # packed per-head scores accumulate on top
                    for j in range(4):
                        qs = slice(32 * j, 32 * (j + 1))
                        moff = mt * 128 + 32 * j
                        for h in range(H):
                            hs = slice(32 * h, 32 * (h + 1))
                            off = NCH * (h % 2)
                            nc.tensor.matmul(
                                sc[h // 2][qs, off:off + NCH],
                                kT_sb[hs, moff:moff + 32],
                                qT_sb[hs, nsl],
                                start=False, stop=(j == 3),
                                tile_position=(32 * h, 32 * j),
                                skip_group_check=True,
                            )
                    th = [thpool.tile([128, 2 * NCH], bf16, tag="th",
                                      name=f"th{p}") for p in range(2)]
                    prev2_tanh = prev_tanh
                    prev_tanh = []
                    for p in range(2):
                        act = nc.scalar.activation(th[p][:], sc[p][:], TANH)
                        prev_tanh.append(act)
                        # head-mean via linearity: av += V'[mt]^T @ tanh_h^T
                        for q in range(2):
                            h = 2 * p + q
                            mm = nc.tensor.matmul(
                                av[:], v_sb[:, mt * 128:(mt + 1) * 128],
                                th[p][:, q * NCH:(q + 1) * NCH],
                                start=(mt == 0 and h == 0),
                                stop=(mt == N_MT - 1 and h == H - 1),
                            )
                    prev_av = [mm]
                oT = opool.tile([D, NCH], f32, tag="oT", bufs=4 * KREP)
                cp = nc.vector.tensor_copy(oT[:], av[:])
                ot_copies.append(cp)
                if rep == KREP - 1:
                    od = nc.gpsimd.dma_start(out=outT[ncg][:], in_=oT[:])
                    tail_insts.extend([cp, od])

            tail_insts.extend(prev_tanh)
            tail_insts.extend(prev_av)
            for ti in tail_insts:
                nz = nc.sync.nop(nofuse=True, hint="predrain")
                add_dep_helper(nz.ins, ti.ins, reason="predrain absorb")

    return nc


def get_nc():
    if "nc" not in _NC_CACHE:
        _NC_CACHE["nc"] = _build_nc()
    return _NC_CACHE["nc"]


def _prep_in_maps(x, cond, attention_mask, Wq, bq, Wk, bk, Wv, bv):
    import ml_dtypes

    bf16 = ml_dtypes.bfloat16
    s = 1.0 / math.sqrt(128.0)

    Wq_s = (np.asarray(Wq, np.float32) * s).astype(bf16)
    Wk_b = np.asarray(Wk, np.float32).astype(bf16)
    Wv4 = (np.asarray(Wv, np.float32) * 0.25).astype(bf16)

    def _row0(vec):
        m = np.zeros((D, D), np.float32)
        m[0, :] = vec
        return m.astype(bf16)

    bq_s = _row0(np.asarray(bq, np.float32) * s)
    bk_b = _row0(np.asarray(bk, np.float32))
    bv4 = _row0(np.asarray(bv, np.float32) * 0.25)
    onesm = np.zeros((D, NCH), np.float32)
    onesm[0, :] = 1.0
    onesm = onesm.astype(bf16)
    eyef = np.eye(D, dtype=np.float32).astype(bf16)

    x = np.asarray(x, np.float32)
    cond = np.asarray(cond, np.float32)
    attention_mask = np.asarray(attention_mask, np.float32)

    in_maps = []
    for i in range(B):
        in_maps.append({
            "xT": np.ascontiguousarray(x[i].T).astype(bf16),
            "condT": np.ascontiguousarray(cond[i].T).astype(bf16),
            "maskT": np.ascontiguousarray(attention_mask[i].T).astype(bf16),
            "Wq": Wq_s, "Wk": Wk_b, "Wv4": Wv4,
            "bq": bq_s, "bk": bk_b, "bv4": bv4,
            "onesm": onesm, "eyef": eyef,
        })
    return in_maps


def run(x, cond, flags, attention_mask, Wq, bq, Wk, bk, Wv, bv,
        trace=False, tmpdir=None):
    """Returns (out [B,N,D] float32, exec_time_ns or None)."""
    from concourse.bass_utils import run_bass_kernel_spmd

    nc = get_nc()
    in_maps = _prep_in_maps(x, cond, attention_mask, Wq, bq, Wk, bk, Wv, bv)
    res = run_bass_kernel_spmd(
        nc, in_maps, core_ids=list(range(B)), trace=trace, tmpdir=tmpdir,
    )
    out = np.stack(
        [np.concatenate([np.asarray(r[f"outT{i}"], np.float32)
                         for i in range(N_NCH)], axis=1).T
         for r in res.results], axis=0
    )
    return out, res.exec_time_ns


def kernel(**inputs):
    out, _ = run(**inputs)
    return out
